# revision 1
# baseline (speedup 1.0000x reference)
"""Trainium2 Bass kernel for nn_AttentionBlock (N=32, T=1024, C=K=V=512).

Strategy: data-parallel over batch N across 8 NeuronCores (4 batches/core),
no collectives. Per batch on-core:
  xT = transpose(x) via bf16 DMA-XBAR transpose (DRAM round trip), produced
       one batch ahead so the chain hides under the previous batch's compute
  qT = Wq^T xT + bq ; kT = Wk^T xT + bk ; v = xT^T Wv + bv     (bf16 matmuls)
  scoresT[s,t] = k q^T (only tiles with t >= s; strict lower-tri masked)
  attnT = exp(scoresT/sqrt(K)) with row sums accumulated in the same
  scalar-engine pass (softmax over the query axis t, per reference)
  vs[s,:] = v[s,:] / rowsum[s]
  attn_out[t,:] = sum_s attnT[s,t] vs[s,:]  (only s-chunks <= t-chunk)
  out = [x, attn_out]

DMA routing: bulk transfers (x loads, weights, outputs) go through gpsimd
SWDGE; the latency-critical xd-write + XBAR-transpose chain owns the SP
HWDGE ring; casts/exp run on the scalar engine.
"""

import contextlib
import math

import numpy as np

import concourse.bass as bass
import concourse.tile as tile
from bass_rust import add_dep_helper
from concourse import bacc, mybir
from concourse.bass_utils import run_bass_kernel_spmd

N, T, C, K, V = 32, 1024, 512, 512, 512
NCORES = 8
NB = N // NCORES  # batches per core
P = 128
CO = C // P  # 4 chunks of contraction dim
KO = K // P  # 4 chunks of qk feature dim
TO = T // P  # 8 chunks of sequence dim
F32 = mybir.dt.float32
BF16 = mybir.dt.bfloat16
SCALE = 1.0 / math.sqrt(K)
NEG = -1.0e9


def _body(nc, tc, x_ext, w_exts, b_exts, out_ext, reps=1):
    ctxs = []

    def pool(name, bufs, space="SBUF"):
        p = tc.tile_pool(name=name, bufs=bufs, space=space)
        ctxs.append(p)
        return p.__enter__()

    consts = pool("consts", 1)
    wstage = pool("wstage", 2)
    xn_pool = pool("xn", 2)
    xbf_pool = pool("xbf", 2)
    xdram_pool = pool("xdram", 2, space="DRAM")
    xt_pool = pool("xt", 2)
    qk_pool = pool("qk", 1)
    at_pool = pool("at", 1)
    small = pool("small", 4)
    ob_pool = pool("ob", 2)
    pp = pool("pp", 6, space="PSUM")
    pwarm = pool("pwarm", 1, space="PSUM")
    pools = (
        xn_pool,
        xbf_pool,
        xdram_pool,
        xt_pool,
        qk_pool,
        at_pool,
        small,
        ob_pool,
        pp,
        pwarm,
    )

    # ---- constants ----
    # maskbias[s_local, t_local]: 0 where t >= s, NEG where t < s
    maskbias = consts.tile([P, P], F32)
    nc.gpsimd.memset(maskbias, 0.0)
    nc.gpsimd.affine_select(
        out=maskbias,
        in_=maskbias,
        compare_op=mybir.AluOpType.is_ge,
        fill=NEG,
        base=0,
        pattern=[[1, P]],  # +1 per t (free)
        channel_multiplier=-1,  # -1 per s (partition); keep where t - s >= 0
    )

    def load_w(name, w_ext, defer_anchor=None):
        stage = wstage.tile([P, CO, 512], F32, tag="wstage", name=f"stage_{name}")
        dma = nc.gpsimd.dma_start(
            out=stage, in_=w_ext.rearrange("(co p) k -> p co k", p=P)
        )
        if defer_anchor is not None:
            add_dep_helper(dma.ins, defer_anchor.ins, reason="defer behind xT chain")
        wbf = consts.tile([P, CO, 512], BF16, tag=f"w_{name}", name=f"w_{name}")
        nc.vector.tensor_copy(out=wbf, in_=stage)
        return wbf

    w_bfs = [None, None, None]
    bq_t = consts.tile([P, KO], F32, tag="bq")
    bk_t = consts.tile([P, KO], F32, tag="bk")
    bv_b = consts.tile([P, V], F32, tag="bv")

    def early_setup():
        w_bfs[0] = load_w("q", w_exts[0])
        nc.gpsimd.dma_start(out=bq_t, in_=b_exts[0].rearrange("(ko p) -> p ko", p=P))

    def late_setup(anchor):
        w_bfs[1] = load_w("k", w_exts[1], anchor)
        w_bfs[2] = load_w("v", w_exts[2], anchor)
        dma = nc.gpsimd.dma_start(
            out=bk_t, in_=b_exts[1].rearrange("(ko p) -> p ko", p=P)
        )
        add_dep_helper(dma.ins, anchor.ins, reason="defer behind xT chain")
        bv_src = bass.AP(
            tensor=b_exts[2].tensor,
            offset=b_exts[2].offset,
            ap=[[0, P]] + list(b_exts[2].ap),
        )
        dma = nc.gpsimd.dma_start(out=bv_b, in_=bv_src)
        add_dep_helper(dma.ins, anchor.ins, reason="defer behind xT chain")

    loop = tc.For_i(0, reps, 1) if reps > 1 else contextlib.nullcontext()
    with loop:
        _batches(
            nc,
            tc,
            x_ext,
            out_ext,
            w_bfs,
            bq_t,
            bk_t,
            bv_b,
            maskbias,
            pools,
            early_setup,
            late_setup,
        )

    for p in reversed(ctxs):
        p.__exit__(None, None, None)


def _batches(
    nc, tc, x_ext, out_ext, w_bfs, bq_t, bk_t, bv_b, maskbias, pools, early_setup,
    late_setup,
):
    (
        xn_pool,
        xbf_pool,
        xdram_pool,
        xt_pool,
        qk_pool,
        at_pool,
        small,
        ob_pool,
        pp,
        pwarm,
    ) = pools

    def xT_stage(n, prev_last_tr):
        """x load -> bf16 cast -> DRAM round trip -> XBAR transpose."""
        x_nat = xn_pool.tile([P, TO, C], F32, tag="x_nat", name=f"x_nat_{n}")
        x_view = x_ext[n].rearrange("(to p) c -> p to c", p=P)
        x_bf = xbf_pool.tile([P, TO, C], BF16, tag="x_bf", name=f"x_bf_{n}")
        xd = xdram_pool.tile([T, C], BF16, tag="xd", name=f"xd_{n}")
        xd_view = xd.rearrange("(to p) c -> p to c", p=P)
        half = TO // 2
        for h in range(2):
            sl = slice(h * half, (h + 1) * half)
            x_load = nc.gpsimd.dma_start(out=x_nat[:, sl, :], in_=x_view[:, sl, :])
            if prev_last_tr is not None:
                add_dep_helper(
                    x_load.ins,
                    prev_last_tr.ins,
                    reason="defer prefetch behind xT chain",
                )
            nc.scalar.copy(out=x_bf[:, sl, :], in_=x_nat[:, sl, :])
            nc.sync.dma_start(out=xd_view[:, sl, :], in_=x_bf[:, sl, :])
        xT = xt_pool.tile([P, CO, T], BF16, tag="xT", name=f"xT_{n}")
        trs = [
            nc.sync.dma_start_transpose(xT[:, co, :], xd[:, P * co : P * (co + 1)])
            for co in range(CO)
        ]
        return x_nat, xT, trs[-1]

    staged = xT_stage(0, None)
    if early_setup is not None:
        early_setup()
        scratch = small.tile([P, 512], F32, tag="warm_rhs", name="warm_rhs")
        nc.vector.memset(scratch, 0.0)
        wpsum = pwarm.tile([P, 512], F32, tag="warm_ps", name="warm_ps")
        nbig, nsmall = 9, 8
        for d in range(nbig):
            nc.tensor.matmul(
                wpsum, lhsT=maskbias, rhs=scratch, start=(d == 0), stop=False
            )
        for d in range(nsmall):
            nc.tensor.matmul(
                wpsum[:, 0:128],
                lhsT=maskbias,
                rhs=scratch[:, 0:128],
                start=False,
                stop=(d == nsmall - 1),
            )
    for n in range(NB):
        x_nat, xT, last_tr = staged
        if n == 0 and late_setup is not None:
            late_setup(last_tr)
            late_setup = None
        if n + 1 < NB:
            staged = xT_stage(n + 1, last_tr)

        # ---- projections ----
        qT = qk_pool.tile([P, KO, T], BF16, tag="qT", name=f"qT_{n}")
        kT = qk_pool.tile([P, KO, T], BF16, tag="kT", name=f"kT_{n}")
        for wbf, b_t, dst, wname in (
            (w_bfs[0], bq_t, qT, "q"),
            (w_bfs[1], bk_t, kT, "k"),
        ):
            for ko in range(KO):
                pss = [
                    pp.tile([P, 512], F32, tag="psA", name=f"psp_{n}_{wname}_{ko}_{th}")
                    for th in range(2)
                ]
                for ci in range(CO):
                    for th in range(2):
                        mm = nc.tensor.matmul(
                            pss[th],
                            lhsT=wbf[:, ci, P * ko : P * (ko + 1)],
                            rhs=xT[:, ci, 512 * th : 512 * (th + 1)],
                            start=(ci == 0),
                            stop=(ci == CO - 1),
                        )
                        if n == 0 and ko == 0 and th == 0 and ci == 0 and dst is qT:
                            add_dep_helper(
                                mm.ins,
                                last_tr.ins,
                                reason="start PE only when xT complete",
                            )
                for th in range(2):
                    nc.vector.tensor_scalar_add(
                        out=dst[:, ko, 512 * th : 512 * (th + 1)],
                        in0=pss[th],
                        scalar1=b_t[:, ko : ko + 1],
                    )
        v_bf = qk_pool.tile([P, TO, V], BF16, tag="v", name=f"v_{n}")
        for so in range(TO):
            ps = pp.tile([P, 512], F32, tag="psA", name=f"psv_{n}_{so}")
            for ci in range(CO):
                nc.tensor.matmul(
                    ps,
                    lhsT=xT[:, ci, P * so : P * (so + 1)],
                    rhs=w_bfs[2][:, ci, :],
                    start=(ci == 0),
                    stop=(ci == CO - 1),
                )
            nc.vector.tensor_tensor(
                out=v_bf[:, so, :], in0=ps, in1=bv_b, op=mybir.AluOpType.add
            )

        # ---- scores + masked softmax over t (free axis) ----
        attnT = at_pool.tile([P, TO, T], BF16, tag="attnT", name=f"attnT_{n}")
        vs = qk_pool.tile([P, TO, V], BF16, tag="vs", name=f"vs_{n}")
        recips = small.tile([P, TO], F32, tag="recips", name=f"recips_{n}")
        first_exp = None
        for i in range(TO):
            segs = []
            for th in range(2):
                seg_lo = max(512 * th, P * i)
                seg_hi = 512 * (th + 1)
                if seg_hi > seg_lo:
                    segs.append((th, seg_lo, seg_hi))
            ps_map = {
                th: pp.tile([P, 512], F32, tag="psA", name=f"pss_{n}_{i}_{th}")[
                    :, : hi - lo
                ]
                for th, lo, hi in segs
            }
            for ko in range(KO):
                for th, lo, hi in segs:
                    nc.tensor.matmul(
                        ps_map[th],
                        lhsT=kT[:, ko, P * i : P * (i + 1)],
                        rhs=qT[:, ko, lo:hi],
                        start=(ko == 0),
                        stop=(ko == KO - 1),
                    )
            parts = []
            for th, seg_lo, seg_hi in segs:
                ps = ps_map[th]
                if seg_lo == P * i:  # segment starts at the diagonal block
                    nc.vector.tensor_tensor(
                        out=ps[:, 0:P],
                        in0=ps[:, 0:P],
                        in1=maskbias,
                        op=mybir.AluOpType.add,
                    )
                acc = small.tile([P, 1], F32, tag="acc", name=f"acc_{n}_{i}_{th}")
                exp_inst = nc.scalar.activation(
                    out=attnT[:, i, seg_lo:seg_hi],
                    in_=ps,
                    func=mybir.ActivationFunctionType.Exp,
                    scale=SCALE,
                    accum_out=acc,
                )
                if first_exp is None:
                    first_exp = exp_inst
                parts.append(acc)
            if len(parts) == 2:
                rsum = small.tile([P, 1], F32, tag="rsum", name=f"rsum_{n}_{i}")
                nc.vector.tensor_add(out=rsum, in0=parts[0], in1=parts[1])
            else:
                rsum = parts[0]
            nc.vector.reciprocal(out=recips[:, i : i + 1], in_=rsum)
            nc.vector.tensor_scalar_mul(
                out=vs[:, i, :], in0=v_bf[:, i, :], scalar1=recips[:, i : i + 1]
            )

        xcopy = nc.scalar.dma_start(
            out=out_ext[n, :, 0:C].rearrange("(to p) c -> p to c", p=P), in_=x_nat
        )
        if n < NB - 1:
            add_dep_helper(
                xcopy.ins, first_exp.ins, reason="defer x-copy behind scores"
            )

        # ---- attn @ v ----
        o_view = out_ext[n, :, C : C + V].rearrange("(to p) c -> p to c", p=P)
        o_f32 = ob_pool.tile([P, TO, V], F32, tag="o", name=f"o_{n}")
        j_order = range(TO - 1, -1, -1) if n == NB - 1 else range(TO)
        for j in j_order:
            ps = pp.tile([P, 512], F32, tag="psA", name=f"psav_{n}_{j}")
            for i in range(j + 1):
                nc.tensor.matmul(
                    ps,
                    lhsT=attnT[:, i, P * j : P * (j + 1)],
                    rhs=vs[:, i, :],
                    start=(i == 0),
                    stop=(i == j),
                )
            nc.vector.tensor_copy(out=o_f32[:, j, :], in_=ps)
            nc.scalar.dma_start(out=o_view[:, j : j + 1, :], in_=o_f32[:, j : j + 1, :])


def build_nc(reps=1):
    nc = bacc.Bacc("TRN2", target_bir_lowering=False, debug=False, num_devices=NCORES)
    x_ext = nc.dram_tensor("x", [NB, T, C], F32, kind="ExternalInput").ap()
    wq = nc.dram_tensor("Wq", [C, K], F32, kind="ExternalInput").ap()
    bq = nc.dram_tensor("bq", [K], F32, kind="ExternalInput").ap()
    wk = nc.dram_tensor("Wk", [C, K], F32, kind="ExternalInput").ap()
    bk = nc.dram_tensor("bk", [K], F32, kind="ExternalInput").ap()
    wv = nc.dram_tensor("Wv", [C, V], F32, kind="ExternalInput").ap()
    bv = nc.dram_tensor("bv", [V], F32, kind="ExternalInput").ap()
    out_ext = nc.dram_tensor("out", [NB, T, C + V], F32, kind="ExternalOutput").ap()

    with tile.TileContext(nc) as tc:
        _body(nc, tc, x_ext, (wq, wk, wv), (bq, bk, bv), out_ext, reps=reps)
    nc.compile()
    return nc


def make_in_maps(x, Wq, bq, Wk, bk, Wv, bv):
    x = np.ascontiguousarray(np.asarray(x, dtype=np.float32))
    return [
        {
            "x": x[NB * i : NB * (i + 1)],
            "Wq": np.asarray(Wq, np.float32),
            "bq": np.asarray(bq, np.float32),
            "Wk": np.asarray(Wk, np.float32),
            "bk": np.asarray(bk, np.float32),
            "Wv": np.asarray(Wv, np.float32),
            "bv": np.asarray(bv, np.float32),
        }
        for i in range(NCORES)
    ]


def kernel(x, Wq, bq, Wk, bk, Wv, bv):
    nc = build_nc()
    in_maps = make_in_maps(x, Wq, bq, Wk, bk, Wv, bv)
    res = run_bass_kernel_spmd(nc, in_maps, list(range(NCORES)))
    return np.concatenate([res.results[i]["out"] for i in range(NCORES)], axis=0)



# revision 5
# speedup vs baseline: 1.3885x; 1.3885x over previous
"""Trainium2 Bass kernel for nn_AttentionBlock (N=32, T=1024, C=K=V=512).

Strategy: data-parallel over batch N across 8 NeuronCores (4 batches/core),
no collectives. All heavy matmuls run as fp8e4 DoubleRow (2 MACs/cell/cycle,
256-deep contraction per pass); accumulation stays fp32 in PSUM. Per batch:
  xT = transpose(x) via bf16 DMA-XBAR transpose (DRAM round trip), produced
       one batch ahead so the chain hides under the previous batch's compute;
       then cast to fp8 (xT8)
  qT = (Wq*64)^T xT (fp8 DR) ; evac (psum/64 + bq) -> qT8 fp8   (same for k)
  v  = xT^T (Wv*64) (fp8 DR) ; evac psum + 64*bv -> v_bf bf16 (holds 64*v)
  scoresT[s,t] = k q^T (fp8 DR; only tiles with t >= s; lower-tri masked)
  attnT8 = exp(scoresT/sqrt(K) - e_i*ln2) fp8, with row sums accumulated in
  the same scalar-engine pass (softmax over the query axis t, per reference).
  e_i is a static per-s-chunk exponent that centers both attnT8 and vs8 in
  fp8 normal range; it cancels exactly: rsum' = rsum*2^-e_i, so
  vs8[s,:] = v_bf[s,:] * (1/64)/rsum'[s] = v[s,:]*2^e_i/rsum[s]  (fp8)
  attn_out[t,:] = sum_s attnT8[s,t] vs8[s,:]  (fp8 DR, s-chunks <= t-chunk)
  out = [x, attn_out]

DMA routing: bulk transfers (x loads, weights, outputs) go through gpsimd
SWDGE; the latency-critical xd-write + XBAR-transpose chain owns the SP
HWDGE ring; casts/exp run on the scalar engine.
"""

import contextlib
import math

import numpy as np

import concourse.bass as bass
import concourse.tile as tile
from bass_rust import add_dep_helper
from concourse import bacc, mybir
from concourse.bass_utils import run_bass_kernel_spmd

N, T, C, K, V = 32, 1024, 512, 512, 512
NCORES = 8
NB = N // NCORES  # batches per core
P = 128
CO = C // P  # 4 chunks of contraction dim
KO = K // P  # 4 chunks of qk feature dim
TO = T // P  # 8 chunks of sequence dim
F32 = mybir.dt.float32
BF16 = mybir.dt.bfloat16
FP8 = mybir.dt.float8e4
DR = mybir.MatmulPerfMode.DoubleRow
SCALE = 1.0 / math.sqrt(K)
NEG = -1.0e9
WSCALE = 64.0  # keep fp8 weights in normal range
LN2 = math.log(2.0)
# static per-s-chunk balancing exponent: colsum[s] ~= 1.056*(T-s)
BAL_E = [
    round(math.log2(1.056 * (T - 128 * i - 64)) / 2.0) for i in range(TO)
]


def _body(nc, tc, x_ext, w_exts, b_exts, out_ext, reps=1):
    ctxs = []

    def pool(name, bufs, space="SBUF"):
        p = tc.tile_pool(name=name, bufs=bufs, space=space)
        ctxs.append(p)
        return p.__enter__()

    consts = pool("consts", 1)
    wstage = pool("wstage", 2)
    xn_pool = pool("xn", 2)
    xbf_pool = pool("xbf", 2)
    xdram_pool = pool("xdram", 2, space="DRAM")
    xt_pool = pool("xt", 2)
    qk_pool = pool("qk", 1)
    at_pool = pool("at", 1)
    small = pool("small", 4)
    ob_pool = pool("ob", 2)
    pp = pool("pp", 6, space="PSUM")
    pwarm = pool("pwarm", 1, space="PSUM")
    pools = (
        xn_pool,
        xbf_pool,
        xdram_pool,
        xt_pool,
        qk_pool,
        at_pool,
        small,
        ob_pool,
        pp,
        pwarm,
    )

    # ---- constants ----
    # maskbias[s_local, t_local]: 0 where t >= s, NEG where t < s
    maskbias = consts.tile([P, P], F32)
    nc.gpsimd.memset(maskbias, 0.0)
    nc.gpsimd.affine_select(
        out=maskbias,
        in_=maskbias,
        compare_op=mybir.AluOpType.is_ge,
        fill=NEG,
        base=0,
        pattern=[[1, P]],  # +1 per t (free)
        channel_multiplier=-1,  # -1 per s (partition); keep where t - s >= 0
    )

    def load_w(name, w_ext, defer_anchor=None):
        stage = wstage.tile([P, CO, 512], F32, tag="wstage", name=f"stage_{name}")
        dma = nc.gpsimd.dma_start(
            out=stage, in_=w_ext.rearrange("(co p) k -> p co k", p=P)
        )
        if defer_anchor is not None:
            add_dep_helper(dma.ins, defer_anchor.ins, reason="defer behind xT chain")
        w8 = consts.tile([P, CO, 512], FP8, tag=f"w_{name}", name=f"w_{name}")
        nc.vector.tensor_scalar_mul(out=w8, in0=stage, scalar1=WSCALE)
        return w8

    w_8s = [None, None, None]
    bq_t = consts.tile([P, KO], F32, tag="bq")
    bk_t = consts.tile([P, KO], F32, tag="bk")
    bv_b = consts.tile([P, V], F32, tag="bv")
    balbias = consts.tile([P, TO], F32, tag="balbias")
    for i in range(TO):
        nc.gpsimd.memset(balbias[:, i : i + 1], -BAL_E[i] * LN2)

    def early_setup():
        w_8s[0] = load_w("q", w_exts[0])
        nc.gpsimd.dma_start(out=bq_t, in_=b_exts[0].rearrange("(ko p) -> p ko", p=P))

    def late_setup(anchor):
        w_8s[1] = load_w("k", w_exts[1], anchor)
        w_8s[2] = load_w("v", w_exts[2], anchor)
        dma = nc.gpsimd.dma_start(
            out=bk_t, in_=b_exts[1].rearrange("(ko p) -> p ko", p=P)
        )
        add_dep_helper(dma.ins, anchor.ins, reason="defer behind xT chain")
        bv_src = bass.AP(
            tensor=b_exts[2].tensor,
            offset=b_exts[2].offset,
            ap=[[0, P]] + list(b_exts[2].ap),
        )
        dma = nc.gpsimd.dma_start(out=bv_b, in_=bv_src)
        add_dep_helper(dma.ins, anchor.ins, reason="defer behind xT chain")
        # v evac adds bv_b directly to the (64x)-scaled psum
        nc.vector.tensor_scalar_mul(out=bv_b, in0=bv_b, scalar1=WSCALE)

    loop = tc.For_i(0, reps, 1) if reps > 1 else contextlib.nullcontext()
    with loop:
        _batches(
            nc,
            tc,
            x_ext,
            out_ext,
            w_8s,
            bq_t,
            bk_t,
            bv_b,
            balbias,
            maskbias,
            pools,
            early_setup,
            late_setup,
        )

    for p in reversed(ctxs):
        p.__exit__(None, None, None)


def _batches(
    nc, tc, x_ext, out_ext, w_8s, bq_t, bk_t, bv_b, balbias, maskbias, pools,
    early_setup, late_setup,
):
    (
        xn_pool,
        xbf_pool,
        xdram_pool,
        xt_pool,
        qk_pool,
        at_pool,
        small,
        ob_pool,
        pp,
        pwarm,
    ) = pools

    def xT_stage(n, prev_last_tr):
        """x load -> bf16 cast -> DRAM round trip -> XBAR transpose -> fp8."""
        x_nat = xn_pool.tile([P, TO, C], F32, tag="x_nat", name=f"x_nat_{n}")
        x_view = x_ext[n].rearrange("(to p) c -> p to c", p=P)
        x_bf = xbf_pool.tile([P, TO, C], BF16, tag="x_bf", name=f"x_bf_{n}")
        xd = xdram_pool.tile([T, C], BF16, tag="xd", name=f"xd_{n}")
        xd_view = xd.rearrange("(to p) c -> p to c", p=P)
        half = TO // 2
        for h in range(2):
            sl = slice(h * half, (h + 1) * half)
            x_load = nc.gpsimd.dma_start(out=x_nat[:, sl, :], in_=x_view[:, sl, :])
            if prev_last_tr is not None:
                add_dep_helper(
                    x_load.ins,
                    prev_last_tr.ins,
                    reason="defer prefetch behind xT chain",
                )
            nc.scalar.copy(out=x_bf[:, sl, :], in_=x_nat[:, sl, :])
            nc.sync.dma_start(out=xd_view[:, sl, :], in_=x_bf[:, sl, :])
        xT = xt_pool.tile([P, CO, T], BF16, tag="xT", name=f"xT_{n}")
        trs = [
            nc.sync.dma_start_transpose(xT[:, co, :], xd[:, P * co : P * (co + 1)])
            for co in range(CO)
        ]
        x8 = xt_pool.tile([P, CO, T], FP8, tag="x8", name=f"x8_{n}")
        for co in range(CO):
            nc.vector.tensor_copy(out=x8[:, co, :], in_=xT[:, co, :])
        return x_nat, x8, trs[-1]

    staged = xT_stage(0, None)
    if early_setup is not None:
        early_setup()
        scratch = small.tile([P, 512], F32, tag="warm_rhs", name="warm_rhs")
        nc.vector.memset(scratch, 0.0)
        wpsum = pwarm.tile([P, 512], F32, tag="warm_ps", name="warm_ps")
        nbig, nsmall = 9, 8
        for d in range(nbig):
            nc.tensor.matmul(
                wpsum, lhsT=maskbias, rhs=scratch, start=(d == 0), stop=False
            )
        for d in range(nsmall):
            nc.tensor.matmul(
                wpsum[:, 0:128],
                lhsT=maskbias,
                rhs=scratch[:, 0:128],
                start=False,
                stop=(d == nsmall - 1),
            )
    for n in range(NB):
        x_nat, x8, last_tr = staged
        if n == 0 and late_setup is not None:
            late_setup(last_tr)
            late_setup = None
        if n + 1 < NB:
            staged = xT_stage(n + 1, last_tr)

        # ---- projections (fp8 DoubleRow over c-chunk pairs) ----
        qT = qk_pool.tile([P, KO, T], FP8, tag="qT", name=f"qT_{n}")
        kT = qk_pool.tile([P, KO, T], FP8, tag="kT", name=f"kT_{n}")
        for wi, b_t, dst, wname in (
            (0, bq_t, qT, "q"),
            (1, bk_t, kT, "k"),
        ):
            for ko in range(KO):
                pss = [
                    pp.tile([P, 512], F32, tag="psA", name=f"psp_{n}_{wname}_{ko}_{th}")
                    for th in range(2)
                ]
                for cm in range(2):
                    for th in range(2):
                        mm = nc.tensor.matmul(
                            pss[th],
                            lhsT=w_8s[wi][:, 2 * cm : 2 * cm + 2, P * ko : P * (ko + 1)],
                            rhs=x8[:, 2 * cm : 2 * cm + 2, 512 * th : 512 * (th + 1)],
                            start=(cm == 0),
                            stop=(cm == 1),
                            perf_mode=DR,
                        )
                        if n == 0 and ko == 0 and th == 0 and cm == 0 and wi == 0:
                            add_dep_helper(
                                mm.ins,
                                last_tr.ins,
                                reason="start PE only when x8 complete",
                            )
                for th in range(2):
                    sl = slice(512 * th, 512 * (th + 1))
                    nc.vector.tensor_scalar(
                        out=dst[:, ko, sl],
                        in0=pss[th],
                        scalar1=1.0 / WSCALE,
                        scalar2=b_t[:, ko : ko + 1],
                        op0=mybir.AluOpType.mult,
                        op1=mybir.AluOpType.add,
                    )
        v_bf = qk_pool.tile([P, TO, V], BF16, tag="v", name=f"v_{n}")
        for so in range(TO):
            ps = pp.tile([P, 512], F32, tag="psA", name=f"psv_{n}_{so}")
            for cm in range(2):
                nc.tensor.matmul(
                    ps,
                    lhsT=x8[:, 2 * cm : 2 * cm + 2, P * so : P * (so + 1)],
                    rhs=w_8s[2][:, 2 * cm : 2 * cm + 2, :],
                    start=(cm == 0),
                    stop=(cm == 1),
                    perf_mode=DR,
                )
            nc.vector.tensor_tensor(
                out=v_bf[:, so, :], in0=ps, in1=bv_b, op=mybir.AluOpType.add
            )

        # ---- scores + masked softmax over t (free axis), fp8 DR ----
        attnT = at_pool.tile([P, TO, T], FP8, tag="attnT", name=f"attnT_{n}")
        vs = qk_pool.tile([P, TO, V], FP8, tag="vs", name=f"vs_{n}")
        recips = small.tile([P, TO], F32, tag="recips", name=f"recips_{n}")
        first_exp = None
        for i in range(TO):
            segs = []
            for th in range(2):
                seg_lo = max(512 * th, P * i)
                seg_hi = 512 * (th + 1)
                if seg_hi > seg_lo:
                    segs.append((th, seg_lo, seg_hi))
            ps_map = {
                th: pp.tile([P, 512], F32, tag="psA", name=f"pss_{n}_{i}_{th}")[
                    :, : hi - lo
                ]
                for th, lo, hi in segs
            }
            for km in range(2):
                for th, lo, hi in segs:
                    nc.tensor.matmul(
                        ps_map[th],
                        lhsT=kT[:, 2 * km : 2 * km + 2, P * i : P * (i + 1)],
                        rhs=qT[:, 2 * km : 2 * km + 2, lo:hi],
                        start=(km == 0),
                        stop=(km == 1),
                        perf_mode=DR,
                    )
            parts = []
            for th, seg_lo, seg_hi in segs:
                ps = ps_map[th]
                if seg_lo == P * i:  # segment starts at the diagonal block
                    nc.vector.tensor_tensor(
                        out=ps[:, 0:P],
                        in0=ps[:, 0:P],
                        in1=maskbias,
                        op=mybir.AluOpType.add,
                    )
                acc = small.tile([P, 1], F32, tag="acc", name=f"acc_{n}_{i}_{th}")
                exp_inst = nc.scalar.activation(
                    out=attnT[:, i, seg_lo:seg_hi],
                    in_=ps,
                    func=mybir.ActivationFunctionType.Exp,
                    scale=SCALE,
                    bias=balbias[:, i : i + 1],
                    accum_out=acc,
                )
                if first_exp is None:
                    first_exp = exp_inst
                parts.append(acc)
            if len(parts) == 2:
                rsum = small.tile([P, 1], F32, tag="rsum", name=f"rsum_{n}_{i}")
                nc.vector.tensor_add(out=rsum, in0=parts[0], in1=parts[1])
            else:
                rsum = parts[0]
            nc.vector.reciprocal(out=recips[:, i : i + 1], in_=rsum)
            nc.vector.tensor_scalar_mul(
                out=recips[:, i : i + 1],
                in0=recips[:, i : i + 1],
                scalar1=1.0 / WSCALE,
            )
            nc.vector.tensor_scalar_mul(
                out=vs[:, i, :], in0=v_bf[:, i, :], scalar1=recips[:, i : i + 1]
            )

        xcopy = nc.scalar.dma_start(
            out=out_ext[n, :, 0:C].rearrange("(to p) c -> p to c", p=P), in_=x_nat
        )
        if n < NB - 1:
            add_dep_helper(
                xcopy.ins, first_exp.ins, reason="defer x-copy behind scores"
            )

        # ---- attn @ v (fp8 DR over s-chunk pairs; odd leftover normal fp8) ----
        o_view = out_ext[n, :, C : C + V].rearrange("(to p) c -> p to c", p=P)
        o_f32 = ob_pool.tile([P, TO, V], F32, tag="o", name=f"o_{n}")
        j_order = range(TO - 1, -1, -1) if n == NB - 1 else range(TO)
        for j in j_order:
            ps = pp.tile([P, 512], F32, tag="psA", name=f"psav_{n}_{j}")
            npair = (j + 1) // 2
            for m in range(npair):
                nc.tensor.matmul(
                    ps,
                    lhsT=attnT[:, 2 * m : 2 * m + 2, P * j : P * (j + 1)],
                    rhs=vs[:, 2 * m : 2 * m + 2, :],
                    start=(m == 0),
                    stop=(m == npair - 1 and (j + 1) % 2 == 0),
                    perf_mode=DR,
                )
            if (j + 1) % 2 == 1:
                nc.tensor.matmul(
                    ps,
                    lhsT=attnT[:, j, P * j : P * (j + 1)],
                    rhs=vs[:, j, :],
                    start=(npair == 0),
                    stop=True,
                )
            nc.vector.tensor_copy(out=o_f32[:, j, :], in_=ps)
            nc.scalar.dma_start(out=o_view[:, j : j + 1, :], in_=o_f32[:, j : j + 1, :])


def build_nc(reps=1):
    nc = bacc.Bacc("TRN2", target_bir_lowering=False, debug=False, num_devices=NCORES)
    x_ext = nc.dram_tensor("x", [NB, T, C], F32, kind="ExternalInput").ap()
    wq = nc.dram_tensor("Wq", [C, K], F32, kind="ExternalInput").ap()
    bq = nc.dram_tensor("bq", [K], F32, kind="ExternalInput").ap()
    wk = nc.dram_tensor("Wk", [C, K], F32, kind="ExternalInput").ap()
    bk = nc.dram_tensor("bk", [K], F32, kind="ExternalInput").ap()
    wv = nc.dram_tensor("Wv", [C, V], F32, kind="ExternalInput").ap()
    bv = nc.dram_tensor("bv", [V], F32, kind="ExternalInput").ap()
    out_ext = nc.dram_tensor("out", [NB, T, C + V], F32, kind="ExternalOutput").ap()

    with tile.TileContext(nc) as tc:
        _body(nc, tc, x_ext, (wq, wk, wv), (bq, bk, bv), out_ext, reps=reps)
    nc.compile()
    return nc


def make_in_maps(x, Wq, bq, Wk, bk, Wv, bv):
    x = np.ascontiguousarray(np.asarray(x, dtype=np.float32))
    return [
        {
            "x": x[NB * i : NB * (i + 1)],
            "Wq": np.asarray(Wq, np.float32),
            "bq": np.asarray(bq, np.float32),
            "Wk": np.asarray(Wk, np.float32),
            "bk": np.asarray(bk, np.float32),
            "Wv": np.asarray(Wv, np.float32),
            "bv": np.asarray(bv, np.float32),
        }
        for i in range(NCORES)
    ]


def kernel(x, Wq, bq, Wk, bk, Wv, bv):
    nc = build_nc()
    in_maps = make_in_maps(x, Wq, bq, Wk, bk, Wv, bv)
    res = run_bass_kernel_spmd(nc, in_maps, list(range(NCORES)))
    return np.concatenate([res.results[i]["out"] for i in range(NCORES)], axis=0)
